# revision 44
# baseline (speedup 1.0000x reference)
"""Bass/Trainium2 kernel for batched 3D FFT circular convolution.

Reference computes: y = Re(IFFT3(FFT3(x) . FFT3(w))) / (N * sqrt(N)) scaling
(net: y = circular_conv3d(x, w) / sqrt(N)), x: (16, 32, 128, 128) f32,
w: (32, 128, 128) f32.

Strategy (pure data parallel over batch, 8 cores x 2 samples):
- Pack two real samples as one complex volume z = x0 + i*x1. Then
  y_pair = IFFT3(FFT3(z) * W~) and y0 = Re, y1 = Im (exact because w real).
- W~ = FFT3(w)/(N*sqrt(N)) is a function of the (per-call constant) filter
  only; precomputed host-side in float64, uploaded in stage-C layout,
  replicated to all cores.
- FFTs as DFT-matrix matmuls on the tensor engine (fp32r). The first and
  last 128-FFT stages are DATA-STATIONARY (block loaded as the stationary
  operand, DFT matrices stream), which transposes partition<->block-col in
  the same pass, eliminating two whole PE transpose stages. The size-32
  axis is transformed with a block-diagonal 4x(32x32) DFT.

Layouts per stage (partition | free), free index given as linear combination:
  L0  [d2 | d1,d3]   f = d1*128 + d3          (natural DMA: 512B runs)
  A'  data-stat FFT d2 (+transpose) -> [d3 | d1,k2]
  B   FFT d3   -> evict-scatter -> [k3 | k2l,k2h,d1]  f = k2l*128 + k2h*32 + d1
  T2  per-k2l transpose -> [(k2h,d1) | k2l,k3]
  C   FFT d1 (block-diag) -> [(k2h,k1) | k2l,k3];  * W~ fused into eviction
  D   IFFT d1  -> [(k2h,d1') | k2l,k3]
  T3  per-k2l transpose, evict-scatter -> [k3 | d1',k2]  f = d1'*128 + k2
  E'  data-stat IFFT d3 (+transpose) -> [k2 | d1',d3']
  F   IFFT d2  -> [d2' | d1',d3'] -> DMA out (chunked per 8-d1 slice)
"""

import numpy as np

D1, D2, D3 = 32, 128, 128
NTOT = D1 * D2 * D3
FREE = D1 * D3  # 4096
B = 16
NCORES = 8

# const matrix slots in the packed (12,128,128) consts input.
# Adjacent pairs used as 256-col moving operands by the data-stationary
# stages: (F2R,F2I) fwd-R, (F2In,F2R) fwd-I, (F2R2,F2In2) inv-R,
# (F2I2,F2R3) inv-I.
(F2R, F2I, F2In, F2R2, F2In2, F2I2, F2R3,
 BDR, BDI, BDIn, IDENT, _PAD) = range(12)
NCONST = 12


def _tf32(a):
    """Round fp32 array to tf32 (10-bit mantissa, round-to-nearest-even)."""
    b = np.ascontiguousarray(a, dtype=np.float32).view(np.uint32)
    r = b + np.uint32(0x00000FFF) + ((b >> np.uint32(13)) & np.uint32(1))
    r &= np.uint32(0xFFFFE000)
    return r.view(np.float32)


def _bf16(a):
    """Round fp32 array to bf16 (round-to-nearest-even)."""
    import ml_dtypes

    return np.ascontiguousarray(a, dtype=np.float32).astype(ml_dtypes.bfloat16)


def _consts_np():
    k = np.arange(128)
    F2 = np.exp(-2j * np.pi * np.outer(k, k) / 128)
    k1 = np.arange(32)
    F1 = np.exp(-2j * np.pi * np.outer(k1, k1) / 32)
    BD = np.zeros((128, 128), complex)
    for g in range(4):
        BD[32 * g:32 * g + 32, 32 * g:32 * g + 32] = F1
    mats = np.stack([
        F2.real, F2.imag, -F2.imag, F2.real, -F2.imag, F2.imag, F2.real,
        BD.real, BD.imag, -BD.imag,
        np.eye(128), np.zeros((128, 128)),
    ])
    # prearranged [128, NCONST*128] so the device DMA is fully contiguous;
    # stage A' additionally gets a tiny bf16 copy of its two moving pairs
    # (slots 0..3) — bf16 enables FWL and halves the x upload
    strip = np.ascontiguousarray(
        mats.transpose(1, 0, 2), dtype=np.float32).reshape(128, NCONST * 128)
    return _tf32(strip), _bf16(strip[:, :4 * 128])


def _what_np(w_real):
    """W~ = FFT3(w)/(N*sqrt(N)) laid out as stage C sees the spectrum:
    partition p = 32*(k2>>5) + k1, free f = 128*(k2&31) + k3."""
    alpha = 1.0 / (NTOT * np.sqrt(np.float64(NTOT)))
    Wc = np.fft.fftn(np.asarray(w_real, dtype=np.float64)) * alpha
    W4 = Wc.reshape(D1, 4, 32, D3).transpose(1, 0, 2, 3).reshape(128, FREE)
    return (np.ascontiguousarray(W4.real, dtype=np.float32),
            np.ascontiguousarray(W4.imag, dtype=np.float32))


def _build_program():
    import concourse.mybir as mybir
    import concourse.tile as tile
    from concourse import bacc

    f32 = mybir.dt.float32
    f32r = mybir.dt.float32r
    bf16 = mybir.dt.bfloat16

    nc = bacc.Bacc("TRN2")
    # x/y are pre/post-transposed on the host to (d2, d1, d3) so every DMA
    # run is >=2KB contiguous (descriptor-rate was the input bottleneck);
    # x is uploaded bf16 (halves the input stream; only stage A' sees it)
    x0_d = nc.dram_tensor("x0", (D2, D1, D3), bf16, kind="ExternalInput")
    x1_d = nc.dram_tensor("x1", (D2, D1, D3), bf16, kind="ExternalInput")
    wr_d = nc.dram_tensor("wr", (128, FREE), f32, kind="ExternalInput")
    wi_d = nc.dram_tensor("wi", (128, FREE), f32, kind="ExternalInput")
    c_d = nc.dram_tensor("consts", (128, NCONST * 128), f32r,
                         kind="ExternalInput")
    cb_d = nc.dram_tensor("consts_bf", (128, 4 * 128), bf16,
                          kind="ExternalInput")
    y0_d = nc.dram_tensor("y0", (D2, D1, D3), f32, kind="ExternalOutput")
    y1_d = nc.dram_tensor("y1", (D2, D1, D3), f32, kind="ExternalOutput")

    with tile.TileContext(nc) as tc:
        with (
            tc.tile_pool(name="sb", bufs=1) as sb,
            tc.tile_pool(name="tp", bufs=3) as tp,
            tc.tile_pool(name="ps", bufs=4, space="PSUM") as ps,
        ):
            consts = sb.tile([128, NCONST * 128], f32r, name="consts")
            consts_bf = sb.tile([128, 4 * 128], bf16, name="consts_bf")
            # A' is gated only on the tiny bf16 strip — issue it first
            nc.sync.dma_start(out=consts_bf, in_=cb_d.ap())

            # PE warm-up: HAM un-throttles the PE clock (1.2 -> 2.4 GHz)
            # only after ~3.4us of sustained matmul activity. Burn that
            # window on throwaway matmuls over the const strip while the
            # input DMAs stream, so stage A' starts at full clock.
            wps = ps.tile([128, 512], f32, name="warm", tag="ps")
            for i in range(8):
                nc.tensor.matmul(wps, consts_bf[:, :128], consts_bf,
                                 start=(i == 0), stop=(i == 7))
            wscr = sb.tile([128, 512], f32, name="wscr")
            nc.vector.tensor_copy(wscr, wps)

            def M(i, n=1):
                return consts[:, i * 128:(i + n) * 128]

            def Mb(i, n=1):
                return consts_bf[:, i * 128:(i + n) * 128]

            xin = [sb.tile([128, FREE], bf16, name=f"xin{c}")
                   for c in range(2)]
            zA = [sb.tile([128, FREE], f32r, name=f"zA{c}") for c in range(2)]
            zB = [sb.tile([128, FREE], f32r, name=f"zB{c}") for c in range(2)]
            wR = sb.tile([128, FREE], f32, name="wR")
            wI = sb.tile([128, FREE], f32, name="wI")

            # input DMAs, chunked along d1 and issued from BOTH HWDGE
            # engines (SP + ACT) in parallel — descriptor generation
            # (DIRECT2D) serializes per engine and can starve stage A'.
            # Both q=0 chunks go on the EARLY engine (sync) so A' block 0
            # (which needs x0 AND x1) is ready as the PE warm-up ends.
            dv0 = xin[0].rearrange("p (a c) -> p a c", a=D1)
            dv1 = xin[1].rearrange("p (a c) -> p a c", a=D1)
            nc.sync.dma_start(out=dv0[:, 0:8, :], in_=x0_d.ap()[:, 0:8, :])
            nc.sync.dma_start(out=dv1[:, 0:8, :], in_=x1_d.ap()[:, 0:8, :])
            for q in range(1, 4):
                nc.sync.dma_start(out=dv0[:, 8 * q:8 * (q + 1), :],
                                  in_=x0_d.ap()[:, 8 * q:8 * (q + 1), :])
                nc.scalar.dma_start(out=dv1[:, 8 * q:8 * (q + 1), :],
                                    in_=x1_d.ap()[:, 8 * q:8 * (q + 1), :])
            nc.scalar.dma_start(out=consts, in_=c_d.ap())
            nc.sync.dma_start(out=wR, in_=wr_d.ap())
            nc.scalar.dma_start(out=wI, in_=wi_d.ap())

            ectr = [0]

            def evict(dst, src):
                # psum->sbuf eviction split between DVE and ACT (3:5 — DVE
                # also carries the 4 stage-C multiplies)
                if ectr[0] % 8 < 3:
                    nc.vector.tensor_copy(dst, src)
                else:
                    nc.scalar.copy(dst, src)
                ectr[0] += 1

            def scatter_dst(buf, kind, t):
                if kind == "B":  # psum enum (d1 8, k2h 4, k2l 32) -> f=k2l*128+k2h*32+d1
                    v = buf.rearrange("p (k2l k2h d1) -> p d1 k2h k2l",
                                      k2l=32, k2h=4, d1=32)
                    return v[:, 8 * t:8 * (t + 1), :, :]
                else:  # T3: psum enum (k2l 8, k2h 4, d1 32) -> f=d1*128+k2h*32+k2l
                    v = buf.rearrange("p (d1 k2h k2l) -> p k2l k2h d1",
                                      d1=32, k2h=4, k2l=32)
                    return v[:, 8 * t:8 * (t + 1), :, :]

            def ds_stage(dst, src, pairR, pairI):
                """Data-stationary FFT along the partition axis, transposing
                partition<->block-col in the same pass.
                out_R = ZR^T pR0 + ZI^T pI0 ; out_I = ZR^T pR1 + ZI^T pI1
                where pairR = [pR0|pR1], pairI = [pI0|pI1] (256-col consts).
                """
                for t in range(8):
                    pt = ps.tile([128, 1024], f32, name="pt", tag="ps")
                    for b in range(4):
                        blk = 4 * t + b
                        o = slice(256 * b, 256 * (b + 1))
                        sl = slice(128 * blk, 128 * (blk + 1))
                        nc.tensor.matmul(pt[:, o], src[0][:, sl], pairR,
                                         start=True, stop=False)
                        nc.tensor.matmul(pt[:, o], src[1][:, sl], pairI,
                                         start=False, stop=True)
                    # psum cols: (blk 4, comp 2, k 128) -> per-comp strided
                    pv = pt.rearrange("p (b two c) -> p two b c", b=4, two=2)
                    for comp in range(2):
                        dv = dst[comp][:, 512 * t:512 * (t + 1)].rearrange(
                            "p (b c) -> p b c", b=4)
                        evict(dv, pv[:, comp])

            def fft_stage(dst, src, mR, mI, mIn, scatter=None, mid=False,
                          grouped=True, evict_eng=None):
                """out_R = mR^T R + mIn^T I ; out_I = mI^T R + mR^T I.
                Matmuls grouped by stationary weight (3 LDW per t)."""
                for t in range(4):
                    pR = ps.tile([128, 1024], f32, name="pR", tag="ps")
                    pI = ps.tile([128, 1024], f32, name="pI", tag="ps")
                    hs = [(slice(1024 * t + 512 * h, 1024 * t + 512 * (h + 1)),
                           slice(512 * h, 512 * (h + 1))) for h in range(2)]
                    if grouped:
                        for s, o in hs:
                            nc.tensor.matmul(pR[:, o], M(mR), src[0][:, s],
                                             start=True, stop=False)
                        for s, o in hs:
                            nc.tensor.matmul(pI[:, o], M(mR), src[1][:, s],
                                             start=True, stop=False)
                        for s, o in hs:
                            nc.tensor.matmul(pI[:, o], M(mI), src[0][:, s],
                                             start=False, stop=True)
                        for s, o in hs:
                            nc.tensor.matmul(pR[:, o], M(mIn), src[1][:, s],
                                             start=False, stop=True)
                    else:
                        for s, o in hs:
                            nc.tensor.matmul(pR[:, o], M(mR), src[0][:, s],
                                             start=True, stop=False)
                            nc.tensor.matmul(pI[:, o], M(mI), src[0][:, s],
                                             start=True, stop=False)
                            nc.tensor.matmul(pR[:, o], M(mIn), src[1][:, s],
                                             start=False, stop=True)
                            nc.tensor.matmul(pI[:, o], M(mR), src[1][:, s],
                                             start=False, stop=True)
                    sl = slice(1024 * t, 1024 * (t + 1))
                    if mid:
                        # fused pointwise: V = Z * W~ straight out of PSUM
                        # (multiplies on DVE, combines on idle GPSIMD; the
                        # final add — which gates stage D — is split
                        # GPSIMD/DVE so its latency halves)
                        t1 = tp.tile([128, 1024], f32, name="t1", tag="t1")
                        t2 = tp.tile([128, 1024], f32, name="t2", tag="t2")
                        nc.vector.tensor_tensor(t1, pR, wR[:, sl],
                                                op=mybir.AluOpType.mult)
                        nc.vector.tensor_tensor(t2, pI, wI[:, sl],
                                                op=mybir.AluOpType.mult)
                        nc.gpsimd.tensor_sub(dst[0][:, sl], t1, t2)
                        t3 = tp.tile([128, 1024], f32, name="t3", tag="t1")
                        t4 = tp.tile([128, 1024], f32, name="t4", tag="t2")
                        nc.vector.tensor_tensor(t3, pR, wI[:, sl],
                                                op=mybir.AluOpType.mult)
                        nc.vector.tensor_tensor(t4, pI, wR[:, sl],
                                                op=mybir.AluOpType.mult)
                        nc.gpsimd.tensor_add(dst[1][:, sl], t3, t4)
                    elif scatter is not None:
                        nc.vector.tensor_copy(
                            scatter_dst(dst[0], scatter, t),
                            pR.rearrange("p (a b c) -> p a b c", a=8, b=4, c=32))
                        nc.scalar.copy(
                            scatter_dst(dst[1], scatter, t),
                            pI.rearrange("p (a b c) -> p a b c", a=8, b=4, c=32))
                    elif evict_eng == "s":
                        # ACT-only eviction (stage D: DVE is congested
                        # with stage C's trailing W~ multiplies)
                        nc.scalar.copy(dst[0][:, sl], pR)
                        nc.scalar.copy(dst[1][:, sl], pI)
                    else:
                        evict(dst[0][:, sl], pR)
                        evict(dst[1][:, sl], pI)

            def t_stage(dst, src, scatter=None):
                """per-128-block PE transposes. src/dst are [R, I] pairs.
                g-outer/comp-inner so the next stage's first chunk (which
                needs both comps of g=0) unblocks after 2/8 evictions."""
                for g in range(4):
                    for comp in range(2):
                        pT = ps.tile([128, 1024], f32r, name="pT", tag="ps")
                        for j in range(8):
                            blk = g * 8 + j
                            nc.tensor.transpose(
                                pT[:, 128 * j:128 * (j + 1)],
                                src[comp][:, 128 * blk:128 * (blk + 1)],
                                M(IDENT))
                        sl = slice(1024 * g, 1024 * (g + 1))
                        if scatter is not None:
                            if (comp + g) % 2 == 0:
                                nc.vector.tensor_copy(
                                    scatter_dst(dst[comp], scatter, g),
                                    pT.rearrange("p (a b c) -> p a b c",
                                                 a=8, b=4, c=32))
                            else:
                                nc.scalar.copy(
                                    scatter_dst(dst[comp], scatter, g),
                                    pT.rearrange("p (a b c) -> p a b c",
                                                 a=8, b=4, c=32))
                        else:
                            evict(dst[comp][:, sl], pT)

            # ---------------- z chain ----------------
            ds_stage(zB, xin, Mb(F2R, 2), Mb(F2In, 2))               # A' (A+T1)
            fft_stage(zA, zB, F2R, F2I, F2In, scatter="B")           # B
            t_stage(zB, zA)                                          # T2
            fft_stage(zA, zB, BDR, BDI, BDIn, mid=True)              # C + pointwise
            fft_stage(zB, zA, BDR, BDIn, BDI)                        # D (inverse)
            t_stage(zA, zB, scatter="T3")                            # T3
            ds_stage(zB, zA, M(F2R2, 2), M(F2I2, 2))                 # E' (E+T4)
            # F (inverse d2) with chunked output DMA per t-slice
            fft_stage(zA, zB, F2R, F2In, F2I)                        # F
            # output DMAs split across both HWDGE engines
            for q in range(4):
                for c, dst_d, eng in ((0, y0_d, nc.sync),
                                      (1, y1_d, nc.scalar)):
                    ov = dst_d.ap()
                    iv = zA[c].bitcast(f32).rearrange("p (a c) -> p a c",
                                                      a=D1)
                    eng.dma_start(out=ov[:, 8 * q:8 * (q + 1), :],
                                  in_=iv[:, 8 * q:8 * (q + 1), :])
    return nc


_CACHE = {}


def _get_program():
    if "nc" not in _CACHE:
        nc = _build_program()
        try:
            if not nc.is_finalized():
                nc.finalize()
        except AttributeError:
            nc.finalize()
        _CACHE["nc"] = nc
    return _CACHE["nc"]


def _run(x, w_real, **kw):
    from concourse.bass_utils import run_bass_kernel_spmd

    nc = _get_program()
    consts, consts_bf = _consts_np()
    # host-side marshalling: (b, d1, d2, d3) -> (b, d2, d1, d3) so device
    # DMAs are contiguous; bf16 halves the input stream
    xt = _bf16(np.asarray(x, dtype=np.float32).transpose(0, 2, 1, 3))
    whr, whi = _what_np(w_real)
    in_maps = []
    for c in range(NCORES):
        in_maps.append({
            "x0": xt[2 * c],
            "x1": xt[2 * c + 1],
            "wr": whr,
            "wi": whi,
            "consts": consts,
            "consts_bf": consts_bf,
        })
    res = run_bass_kernel_spmd(nc, in_maps, core_ids=list(range(NCORES)), **kw)
    out = np.empty((B, D1, D2, D3), dtype=np.float32)
    for c in range(NCORES):
        out[2 * c] = res.results[c]["y0"].transpose(1, 0, 2)
        out[2 * c + 1] = res.results[c]["y1"].transpose(1, 0, 2)
    return out, res


def kernel(x: np.ndarray, w_real: np.ndarray) -> np.ndarray:
    return _run(x, w_real)[0]


def kernel_traced(x: np.ndarray, w_real: np.ndarray):
    return _run(x, w_real, trace=True)


# revision 45
# speedup vs baseline: 1.0180x; 1.0180x over previous
"""Bass/Trainium2 kernel for batched 3D FFT circular convolution.

Reference computes: y = Re(IFFT3(FFT3(x) . FFT3(w))) / (N * sqrt(N)) scaling
(net: y = circular_conv3d(x, w) / sqrt(N)), x: (16, 32, 128, 128) f32,
w: (32, 128, 128) f32.

Strategy (pure data parallel over batch, 8 cores x 2 samples):
- Pack two real samples as one complex volume z = x0 + i*x1. Then
  y_pair = IFFT3(FFT3(z) * W~) and y0 = Re, y1 = Im (exact because w real).
- W~ = FFT3(w)/(N*sqrt(N)) is a function of the (per-call constant) filter
  only; precomputed host-side in float64, uploaded in stage-C layout,
  replicated to all cores.
- FFTs as DFT-matrix matmuls on the tensor engine (fp32r). The first and
  last 128-FFT stages are DATA-STATIONARY (block loaded as the stationary
  operand, DFT matrices stream), which transposes partition<->block-col in
  the same pass, eliminating two whole PE transpose stages. The size-32
  axis is transformed with a block-diagonal 4x(32x32) DFT.

Layouts per stage (partition | free), free index given as linear combination:
  L0  [d2 | d1,d3]   f = d1*128 + d3          (natural DMA: 512B runs)
  A'  data-stat FFT d2 (+transpose) -> [d3 | d1,k2]
  B   FFT d3   -> evict-scatter -> [k3 | k2l,k2h,d1]  f = k2l*128 + k2h*32 + d1
  T2  per-k2l transpose -> [(k2h,d1) | k2l,k3]
  C   FFT d1 (block-diag) -> [(k2h,k1) | k2l,k3];  * W~ fused into eviction
  D   IFFT d1  -> [(k2h,d1') | k2l,k3]
  T3  per-k2l transpose, evict-scatter -> [k3 | d1',k2]  f = d1'*128 + k2
  E'  data-stat IFFT d3 (+transpose) -> [k2 | d1',d3']
  F   IFFT d2  -> [d2' | d1',d3'] -> DMA out (chunked per 8-d1 slice)
"""

import numpy as np

D1, D2, D3 = 32, 128, 128
NTOT = D1 * D2 * D3
FREE = D1 * D3  # 4096
B = 16
NCORES = 8

# const matrix slots in the packed (12,128,128) consts input.
# Adjacent pairs used as 256-col moving operands by the data-stationary
# stages: (F2R,F2I) fwd-R, (F2In,F2R) fwd-I, (F2R2,F2In2) inv-R,
# (F2I2,F2R3) inv-I.
(F2R, F2I, F2In, F2R2, F2In2, F2I2, F2R3,
 BDR, BDI, BDIn, IDENT, _PAD) = range(12)
NCONST = 12


def _tf32(a):
    """Round fp32 array to tf32 (10-bit mantissa, round-to-nearest-even)."""
    b = np.ascontiguousarray(a, dtype=np.float32).view(np.uint32)
    r = b + np.uint32(0x00000FFF) + ((b >> np.uint32(13)) & np.uint32(1))
    r &= np.uint32(0xFFFFE000)
    return r.view(np.float32)


def _bf16(a):
    """Round fp32 array to bf16 (round-to-nearest-even)."""
    import ml_dtypes

    return np.ascontiguousarray(a, dtype=np.float32).astype(ml_dtypes.bfloat16)


def _consts_np():
    k = np.arange(128)
    F2 = np.exp(-2j * np.pi * np.outer(k, k) / 128)
    k1 = np.arange(32)
    F1 = np.exp(-2j * np.pi * np.outer(k1, k1) / 32)
    BD = np.zeros((128, 128), complex)
    for g in range(4):
        BD[32 * g:32 * g + 32, 32 * g:32 * g + 32] = F1
    mats = np.stack([
        F2.real, F2.imag, -F2.imag, F2.real, -F2.imag, F2.imag, F2.real,
        BD.real, BD.imag, -BD.imag,
        np.eye(128), np.zeros((128, 128)),
    ])
    # prearranged [128, NCONST*128] so the device DMA is fully contiguous;
    # stage A' additionally gets a tiny bf16 copy of its two moving pairs
    # (slots 0..3) — bf16 enables FWL and halves the x upload
    strip = np.ascontiguousarray(
        mats.transpose(1, 0, 2), dtype=np.float32).reshape(128, NCONST * 128)
    return _tf32(strip), _bf16(strip[:, :4 * 128])


def _what_np(w_real):
    """W~ = FFT3(w)/(N*sqrt(N)) laid out as stage C sees the spectrum:
    partition p = 32*(k2>>5) + k1, free f = 128*(k2&31) + k3."""
    alpha = 1.0 / (NTOT * np.sqrt(np.float64(NTOT)))
    Wc = np.fft.fftn(np.asarray(w_real, dtype=np.float64)) * alpha
    W4 = Wc.reshape(D1, 4, 32, D3).transpose(1, 0, 2, 3).reshape(128, FREE)
    return (np.ascontiguousarray(W4.real, dtype=np.float32),
            np.ascontiguousarray(W4.imag, dtype=np.float32))


def _build_program():
    import concourse.mybir as mybir
    import concourse.tile as tile
    from concourse import bacc

    f32 = mybir.dt.float32
    f32r = mybir.dt.float32r
    bf16 = mybir.dt.bfloat16

    nc = bacc.Bacc("TRN2")
    # x/y are pre/post-transposed on the host to (d2, d1, d3) so every DMA
    # run is >=2KB contiguous (descriptor-rate was the input bottleneck);
    # x is uploaded bf16 (halves the input stream; only stage A' sees it)
    x0_d = nc.dram_tensor("x0", (D2, D1, D3), bf16, kind="ExternalInput")
    x1_d = nc.dram_tensor("x1", (D2, D1, D3), bf16, kind="ExternalInput")
    wr_d = nc.dram_tensor("wr", (128, FREE), f32, kind="ExternalInput")
    wi_d = nc.dram_tensor("wi", (128, FREE), f32, kind="ExternalInput")
    c_d = nc.dram_tensor("consts", (128, NCONST * 128), f32r,
                         kind="ExternalInput")
    cb_d = nc.dram_tensor("consts_bf", (128, 4 * 128), bf16,
                          kind="ExternalInput")
    y0_d = nc.dram_tensor("y0", (D2, D1, D3), f32, kind="ExternalOutput")
    y1_d = nc.dram_tensor("y1", (D2, D1, D3), f32, kind="ExternalOutput")

    with tile.TileContext(nc) as tc:
        with (
            tc.tile_pool(name="sb", bufs=1) as sb,
            tc.tile_pool(name="tp", bufs=3) as tp,
            tc.tile_pool(name="ps", bufs=4, space="PSUM") as ps,
        ):
            consts = sb.tile([128, NCONST * 128], f32r, name="consts")
            consts_bf = sb.tile([128, 4 * 128], bf16, name="consts_bf")
            # A' is gated only on the tiny bf16 strip — issue it first
            nc.sync.dma_start(out=consts_bf, in_=cb_d.ap())

            # PE warm-up: HAM un-throttles the PE clock (1.2 -> 2.4 GHz)
            # only after ~3.4us of sustained matmul activity. Burn that
            # window on throwaway matmuls over the const strip while the
            # input DMAs stream, so stage A' starts at full clock.
            wps = ps.tile([128, 512], f32, name="warm", tag="ps")
            for i in range(8):
                nc.tensor.matmul(wps, consts_bf[:, :128], consts_bf,
                                 start=(i == 0), stop=(i == 7))
            wscr = sb.tile([128, 512], f32, name="wscr")
            nc.vector.tensor_copy(wscr, wps)

            def M(i, n=1):
                return consts[:, i * 128:(i + n) * 128]

            def Mb(i, n=1):
                return consts_bf[:, i * 128:(i + n) * 128]

            xin = [sb.tile([128, FREE], bf16, name=f"xin{c}")
                   for c in range(2)]
            zA = [sb.tile([128, FREE], f32r, name=f"zA{c}") for c in range(2)]
            zB = [sb.tile([128, FREE], f32r, name=f"zB{c}") for c in range(2)]
            wR = sb.tile([128, FREE], f32, name="wR")
            wI = sb.tile([128, FREE], f32, name="wI")

            # input DMAs, chunked along d1 and issued from BOTH HWDGE
            # engines (SP + ACT) in parallel — descriptor generation
            # (DIRECT2D) serializes per engine and can starve stage A'
            for q in range(4):
                dv0 = xin[0].rearrange("p (a c) -> p a c", a=D1)
                dv1 = xin[1].rearrange("p (a c) -> p a c", a=D1)
                nc.sync.dma_start(out=dv0[:, 8 * q:8 * (q + 1), :],
                                  in_=x0_d.ap()[:, 8 * q:8 * (q + 1), :])
                nc.scalar.dma_start(out=dv1[:, 8 * q:8 * (q + 1), :],
                                    in_=x1_d.ap()[:, 8 * q:8 * (q + 1), :])
            nc.scalar.dma_start(out=consts, in_=c_d.ap())
            nc.sync.dma_start(out=wR, in_=wr_d.ap())
            nc.scalar.dma_start(out=wI, in_=wi_d.ap())

            ectr = [0]

            def evict(dst, src):
                # psum->sbuf eviction split between DVE and ACT (3:5 — DVE
                # also carries the 4 stage-C multiplies)
                if ectr[0] % 8 < 3:
                    nc.vector.tensor_copy(dst, src)
                else:
                    nc.scalar.copy(dst, src)
                ectr[0] += 1

            def scatter_dst(buf, kind, t):
                if kind == "B":  # psum enum (d1 8, k2h 4, k2l 32) -> f=k2l*128+k2h*32+d1
                    v = buf.rearrange("p (k2l k2h d1) -> p d1 k2h k2l",
                                      k2l=32, k2h=4, d1=32)
                    return v[:, 8 * t:8 * (t + 1), :, :]
                else:  # T3: psum enum (k2l 8, k2h 4, d1 32) -> f=d1*128+k2h*32+k2l
                    v = buf.rearrange("p (d1 k2h k2l) -> p k2l k2h d1",
                                      d1=32, k2h=4, k2l=32)
                    return v[:, 8 * t:8 * (t + 1), :, :]

            def ds_stage(dst, src, pairR, pairI):
                """Data-stationary FFT along the partition axis, transposing
                partition<->block-col in the same pass.
                out_R = ZR^T pR0 + ZI^T pI0 ; out_I = ZR^T pR1 + ZI^T pI1
                where pairR = [pR0|pR1], pairI = [pI0|pI1] (256-col consts).
                """
                for t in range(8):
                    pt = ps.tile([128, 1024], f32, name="pt", tag="ps")
                    for b in range(4):
                        blk = 4 * t + b
                        o = slice(256 * b, 256 * (b + 1))
                        sl = slice(128 * blk, 128 * (blk + 1))
                        nc.tensor.matmul(pt[:, o], src[0][:, sl], pairR,
                                         start=True, stop=False)
                        nc.tensor.matmul(pt[:, o], src[1][:, sl], pairI,
                                         start=False, stop=True)
                    # psum cols: (blk 4, comp 2, k 128) -> per-comp strided
                    pv = pt.rearrange("p (b two c) -> p two b c", b=4, two=2)
                    for comp in range(2):
                        dv = dst[comp][:, 512 * t:512 * (t + 1)].rearrange(
                            "p (b c) -> p b c", b=4)
                        evict(dv, pv[:, comp])

            def fft_stage(dst, src, mR, mI, mIn, scatter=None, mid=False,
                          grouped=True, evict_eng=None):
                """out_R = mR^T R + mIn^T I ; out_I = mI^T R + mR^T I.
                Matmuls grouped by stationary weight (3 LDW per t)."""
                for t in range(4):
                    pR = ps.tile([128, 1024], f32, name="pR", tag="ps")
                    pI = ps.tile([128, 1024], f32, name="pI", tag="ps")
                    hs = [(slice(1024 * t + 512 * h, 1024 * t + 512 * (h + 1)),
                           slice(512 * h, 512 * (h + 1))) for h in range(2)]
                    if grouped:
                        for s, o in hs:
                            nc.tensor.matmul(pR[:, o], M(mR), src[0][:, s],
                                             start=True, stop=False)
                        for s, o in hs:
                            nc.tensor.matmul(pI[:, o], M(mR), src[1][:, s],
                                             start=True, stop=False)
                        for s, o in hs:
                            nc.tensor.matmul(pI[:, o], M(mI), src[0][:, s],
                                             start=False, stop=True)
                        for s, o in hs:
                            nc.tensor.matmul(pR[:, o], M(mIn), src[1][:, s],
                                             start=False, stop=True)
                    else:
                        for s, o in hs:
                            nc.tensor.matmul(pR[:, o], M(mR), src[0][:, s],
                                             start=True, stop=False)
                            nc.tensor.matmul(pI[:, o], M(mI), src[0][:, s],
                                             start=True, stop=False)
                            nc.tensor.matmul(pR[:, o], M(mIn), src[1][:, s],
                                             start=False, stop=True)
                            nc.tensor.matmul(pI[:, o], M(mR), src[1][:, s],
                                             start=False, stop=True)
                    sl = slice(1024 * t, 1024 * (t + 1))
                    if mid:
                        # fused pointwise: V = Z * W~ straight out of PSUM
                        # (multiplies on DVE, combines on idle GPSIMD; the
                        # final add — which gates stage D — is split
                        # GPSIMD/DVE so its latency halves)
                        t1 = tp.tile([128, 1024], f32, name="t1", tag="t1")
                        t2 = tp.tile([128, 1024], f32, name="t2", tag="t2")
                        nc.vector.tensor_tensor(t1, pR, wR[:, sl],
                                                op=mybir.AluOpType.mult)
                        nc.vector.tensor_tensor(t2, pI, wI[:, sl],
                                                op=mybir.AluOpType.mult)
                        nc.gpsimd.tensor_sub(dst[0][:, sl], t1, t2)
                        t3 = tp.tile([128, 1024], f32, name="t3", tag="t1")
                        t4 = tp.tile([128, 1024], f32, name="t4", tag="t2")
                        nc.vector.tensor_tensor(t3, pR, wI[:, sl],
                                                op=mybir.AluOpType.mult)
                        nc.vector.tensor_tensor(t4, pI, wR[:, sl],
                                                op=mybir.AluOpType.mult)
                        nc.gpsimd.tensor_add(dst[1][:, sl], t3, t4)
                    elif scatter is not None:
                        nc.vector.tensor_copy(
                            scatter_dst(dst[0], scatter, t),
                            pR.rearrange("p (a b c) -> p a b c", a=8, b=4, c=32))
                        nc.scalar.copy(
                            scatter_dst(dst[1], scatter, t),
                            pI.rearrange("p (a b c) -> p a b c", a=8, b=4, c=32))
                    elif evict_eng == "s":
                        # ACT-only eviction (stage D: DVE is congested
                        # with stage C's trailing W~ multiplies)
                        nc.scalar.copy(dst[0][:, sl], pR)
                        nc.scalar.copy(dst[1][:, sl], pI)
                    else:
                        evict(dst[0][:, sl], pR)
                        evict(dst[1][:, sl], pI)

            def t_stage(dst, src, scatter=None):
                """per-128-block PE transposes. src/dst are [R, I] pairs.
                g-outer/comp-inner so the next stage's first chunk (which
                needs both comps of g=0) unblocks after 2/8 evictions."""
                for g in range(4):
                    for comp in range(2):
                        pT = ps.tile([128, 1024], f32r, name="pT", tag="ps")
                        for j in range(8):
                            blk = g * 8 + j
                            nc.tensor.transpose(
                                pT[:, 128 * j:128 * (j + 1)],
                                src[comp][:, 128 * blk:128 * (blk + 1)],
                                M(IDENT))
                        sl = slice(1024 * g, 1024 * (g + 1))
                        if scatter is not None:
                            if (comp + g) % 2 == 0:
                                nc.vector.tensor_copy(
                                    scatter_dst(dst[comp], scatter, g),
                                    pT.rearrange("p (a b c) -> p a b c",
                                                 a=8, b=4, c=32))
                            else:
                                nc.scalar.copy(
                                    scatter_dst(dst[comp], scatter, g),
                                    pT.rearrange("p (a b c) -> p a b c",
                                                 a=8, b=4, c=32))
                        else:
                            evict(dst[comp][:, sl], pT)

            # ---------------- z chain ----------------
            ds_stage(zB, xin, Mb(F2R, 2), Mb(F2In, 2))               # A' (A+T1)
            fft_stage(zA, zB, F2R, F2I, F2In, scatter="B")           # B
            t_stage(zB, zA)                                          # T2
            fft_stage(zA, zB, BDR, BDI, BDIn, mid=True)              # C + pointwise
            fft_stage(zB, zA, BDR, BDIn, BDI)                        # D (inverse)
            t_stage(zA, zB, scatter="T3")                            # T3
            ds_stage(zB, zA, M(F2R2, 2), M(F2I2, 2))                 # E' (E+T4)
            # F (inverse d2) with chunked output DMA per t-slice
            fft_stage(zA, zB, F2R, F2In, F2I)                        # F
            # output DMAs split across both HWDGE engines
            for q in range(4):
                for c, dst_d, eng in ((0, y0_d, nc.sync),
                                      (1, y1_d, nc.scalar)):
                    ov = dst_d.ap()
                    iv = zA[c].bitcast(f32).rearrange("p (a c) -> p a c",
                                                      a=D1)
                    eng.dma_start(out=ov[:, 8 * q:8 * (q + 1), :],
                                  in_=iv[:, 8 * q:8 * (q + 1), :])
    return nc


_CACHE = {}


def _get_program():
    if "nc" not in _CACHE:
        nc = _build_program()
        try:
            if not nc.is_finalized():
                nc.finalize()
        except AttributeError:
            nc.finalize()
        _CACHE["nc"] = nc
    return _CACHE["nc"]


def _run(x, w_real, **kw):
    from concourse.bass_utils import run_bass_kernel_spmd

    nc = _get_program()
    consts, consts_bf = _consts_np()
    # host-side marshalling: (b, d1, d2, d3) -> (b, d2, d1, d3) so device
    # DMAs are contiguous; bf16 halves the input stream
    xt = _bf16(np.asarray(x, dtype=np.float32).transpose(0, 2, 1, 3))
    whr, whi = _what_np(w_real)
    in_maps = []
    for c in range(NCORES):
        in_maps.append({
            "x0": xt[2 * c],
            "x1": xt[2 * c + 1],
            "wr": whr,
            "wi": whi,
            "consts": consts,
            "consts_bf": consts_bf,
        })
    res = run_bass_kernel_spmd(nc, in_maps, core_ids=list(range(NCORES)), **kw)
    out = np.empty((B, D1, D2, D3), dtype=np.float32)
    for c in range(NCORES):
        out[2 * c] = res.results[c]["y0"].transpose(1, 0, 2)
        out[2 * c + 1] = res.results[c]["y1"].transpose(1, 0, 2)
    return out, res


def kernel(x: np.ndarray, w_real: np.ndarray) -> np.ndarray:
    return _run(x, w_real)[0]


def kernel_traced(x: np.ndarray, w_real: np.ndarray):
    return _run(x, w_real, trace=True)
